# revision 5
# baseline (speedup 1.0000x reference)
"""CP tensor-regression-layer kernel for Trainium2 (8 NeuronCores).

Computation (matches the reference einsum pair):
    t[b, r]  = sum_{i,j,k} x[b,i,j,k] * f0[i,r] * f1[j,r] * f2[k,r]
    out[b,c] = sum_r t[b,r] * weight[r] * f3[c,r] + bias[0]

Strategy: data-parallel over the batch dim (32 batches per core, CP
factors replicated).  Per core the big contraction is restructured as
    z[r, b, k] = sum_{ij} (f0[i,r]*f1[j,r]*weight[r]) * x[b, ij, k]
which is a K=2304 matmul against the Khatri-Rao product of f0 and f1,
run as 18 K-chunks of 128 partitions at full PE rate (float32r).  The
remaining k-contraction against f2 runs on the vector engine, and the
class projection against f3^T is one small matmul.  x is pre-permuted
on the host so every DMA is 128 partitions x 6 KiB contiguous runs —
the kernel is HBM-bandwidth bound on loading x (~14.2 MB/core).
"""

import os

import numpy as np

_B, _M1, _M2, _M3, _C, _R = 256, 48, 48, 48, 1000, 64
_NCORES = 8
_BL = _B // _NCORES          # 32 batches per core
_IJ = _M1 * _M2              # 2304 contraction size (i,j fused)
_NCH = _IJ // 128            # 18 K-chunks of 128 partitions
_KB = _BL * _M3              # 1536 moving columns (b,k fused)
_SL = 512                    # matmul slice width (one PSUM bank, fp32)

_cache = {}


def _split_excess_waits(nc, mybir, max_waits=1):
    """Walrus in this container rejects >1 sync-wait per instruction
    ("Too many sync wait commands").  Move excess waits onto chained
    NoOps inserted just before the offending instruction (same engine,
    so program order preserves the gating)."""
    for bb in nc.m.functions[0].blocks:
        insts = bb.instructions
        i = 0
        while i < len(insts):
            inst = insts[i]
            si = getattr(inst, "sync_info", None)
            waits = list(si.on_wait) if si is not None and si.on_wait else []
            if len(waits) > max_waits:
                rest, keep = waits[:-max_waits], waits[-max_waits:]
                pos = i
                for j in range(0, len(rest), max_waits):
                    nop = mybir.InstNoOp(
                        name=f"I-waitsplit-{nc.next_id()}",
                        engine=inst.engine,
                        ins=[],
                        outs=[],
                        sync_info=mybir.SyncInfo(
                            on_wait=list(rest[j : j + max_waits]), on_update=[]
                        ),
                    )
                    nc.register_instruction(nop)
                    insts.insert(pos, nop)
                    pos += 1
                    i += 1
                si.on_wait = keep
            i += 1


def _bcast(ap, bass, shape3):
    """AP broadcast helper: make a 3D view with a stride-0 middle dim."""
    try:
        return ap.unsqueeze(1).broadcast_to(shape3)
    except Exception:
        a = ap.ap
        return bass.AP(
            tensor=ap.tensor,
            offset=ap.offset,
            ap=[list(a[0]), [0, shape3[1]], list(a[1])],
        )


def _build_program():
    import concourse.bass as bass
    import concourse.tile as tile
    from concourse import mybir

    f32 = mybir.dt.float32
    f32r = mybir.dt.float32r

    nc = bass.Bass("TRN2", target_bir_lowering=False, debug=False,
                   num_devices=_NCORES)

    x_d = nc.dram_tensor("x", [128, _NCH, _BL, _M3], f32r, kind="ExternalInput")
    f0t_d = nc.dram_tensor("f0t", [_R, _M1], f32, kind="ExternalInput")
    f1t_d = nc.dram_tensor("f1t", [_R, _M2], f32, kind="ExternalInput")
    f2t_d = nc.dram_tensor("f2t", [_R, _M3], f32, kind="ExternalInput")
    f3t_d = nc.dram_tensor("f3t", [_R, _C], f32, kind="ExternalInput")
    w_d = nc.dram_tensor("w", [_R, 1], f32, kind="ExternalInput")
    b_d = nc.dram_tensor("b", [1, 1], f32, kind="ExternalInput")
    out_d = nc.dram_tensor("out", [_BL, _C], f32, kind="ExternalOutput")
    ident_d = nc.inline_tensor(np.eye(_R, dtype=np.float32), name="ident64")

    with tile.TileContext(nc) as tc:
        with (
            tc.tile_pool(name="consts", bufs=1) as consts,
            tc.tile_pool(name="xp", bufs=4) as xp,
            tc.tile_pool(name="work", bufs=1) as work,
            tc.tile_pool(name="pz", bufs=1, space=bass.MemorySpace.PSUM) as pz,
            tc.tile_pool(name="pt", bufs=2, space=bass.MemorySpace.PSUM) as pt,
            tc.tile_pool(name="po", bufs=1, space=bass.MemorySpace.PSUM) as po,
        ):
            # ---- constants / factor prep (overlaps the x DMA stream) ----
            f0t = consts.tile([_R, _M1], f32)
            nc.sync.dma_start(out=f0t[:], in_=f0t_d[:])
            f1t = consts.tile([_R, _M2], f32)
            nc.sync.dma_start(out=f1t[:], in_=f1t_d[:])
            f2t = consts.tile([_R, _M3], f32)
            nc.sync.dma_start(out=f2t[:], in_=f2t_d[:])
            f3t = consts.tile([_R, _C], f32)
            nc.sync.dma_start(out=f3t[:], in_=f3t_d[:])
            wsb = consts.tile([_R, 1], f32)
            nc.gpsimd.dma_start(out=wsb[:], in_=w_d[:])
            idn = consts.tile([_R, _R], f32)
            nc.sync.dma_start(out=idn[:], in_=ident_d[:])
            bsb = consts.tile([_BL, 1], f32)
            b_ap = b_d[:]
            nc.gpsimd.dma_start(
                out=bsb[:],
                in_=bass.AP(tensor=b_ap.tensor, offset=b_ap.offset,
                            ap=[[0, _BL], [0, 1]]),
            )

            # KR01T[r, i, j] = f0[i,r] * f1[j,r] * weight[r]
            f1tw = consts.tile([_R, _M2], f32)
            nc.vector.tensor_scalar_mul(f1tw[:], f1t[:], wsb[:])
            krt = consts.tile([_R, _M1, _M2], f32)
            in0 = f0t[:].unsqueeze(2).broadcast_to((_R, _M1, _M2))
            in1 = _bcast(f1tw[:], bass, (_R, _M1, _M2))
            nc.vector.tensor_mul(krt[:], in0, in1)

            # Transpose KR to put ij on partitions: kr[p, m, r] = KR[128m+p, r]
            kr = consts.tile([128, _NCH, _R], f32r)
            krt_flat = krt[:].rearrange("r i j -> r (i j)")
            for m in range(_NCH):
                pkr = pt.tile([128, _R], f32)
                nc.tensor.transpose(
                    pkr[:], krt_flat[:, m * 128 : (m + 1) * 128], idn[:]
                )
                nc.vector.tensor_copy(kr[:, m, :], pkr[:])

            # ---- main contraction: z[r, (b,k)] = sum_ij KR[ij,r] x[b,ij,k] ----
            z = pz.tile([_R, _KB], f32)
            for m in range(_NCH):
                xm = xp.tile([128, _BL, _M3], f32r, tag="x")
                nc.sync.dma_start(out=xm[:], in_=x_d[:, m])
                xm_f = xm[:].rearrange("p b k -> p (b k)")
                for s in range(_KB // _SL):
                    nc.tensor.matmul(
                        z[:, s * _SL : (s + 1) * _SL],
                        lhsT=kr[:, m, :],
                        rhs=xm_f[:, s * _SL : (s + 1) * _SL],
                        start=(m == 0),
                        stop=(m == _NCH - 1),
                    )

            # ---- k-contraction against f2 on the vector engine ----
            zf = work.tile([_R, _BL, _M3], f32)
            z3 = z[:].rearrange("r (b k) -> r b k", k=_M3)
            f2b = _bcast(f2t[:], bass, (_R, _BL, _M3))
            nc.vector.tensor_mul(zf[:], z3, f2b)
            tsb = work.tile([_R, _BL], f32)
            nc.vector.reduce_sum(tsb[:], zf[:], axis=mybir.AxisListType.X)

            # ---- class projection: out[b, c] = sum_r t[r,b] f3T[r,c] + bias ----
            op = po.tile([_BL, _C], f32)
            for n0, n1 in ((0, _SL), (_SL, _C)):
                nc.tensor.matmul(
                    op[:, n0:n1],
                    lhsT=tsb[:],
                    rhs=f3t[:, n0:n1],
                    start=True,
                    stop=True,
                )
            osb = work.tile([_BL, _C], f32)
            nc.vector.tensor_scalar_add(osb[:], op[:], bsb[:])
            nc.sync.dma_start(out=out_d[:], in_=osb[:])

    _split_excess_waits(nc, mybir)
    return nc


def _get_program():
    if "nc" not in _cache:
        _cache["nc"] = _build_program()
    return _cache["nc"]


def _host_prep(x, weight, f0, f1, f2, f3, bias):
    """Shard x over cores (batch dim) in a DMA-friendly layout, and
    transpose the small factor matrices (layout only, plus reshapes)."""
    x = np.ascontiguousarray(np.asarray(x, dtype=np.float32))
    f0t = np.ascontiguousarray(np.asarray(f0, np.float32).T)
    f1t = np.ascontiguousarray(np.asarray(f1, np.float32).T)
    f2t = np.ascontiguousarray(np.asarray(f2, np.float32).T)
    f3t = np.ascontiguousarray(np.asarray(f3, np.float32).T)
    w = np.ascontiguousarray(np.asarray(weight, np.float32).reshape(_R, 1))
    b = np.ascontiguousarray(np.asarray(bias, np.float32).reshape(1, 1))
    in_maps = []
    for c in range(_NCORES):
        xc = x[c * _BL : (c + 1) * _BL]
        # [b, ij, k] -> [p, m, b, k] with ij = 128*m + p
        xd = np.ascontiguousarray(
            xc.reshape(_BL, _NCH, 128, _M3).transpose(2, 1, 0, 3)
        )
        in_maps.append(
            {"x": xd, "f0t": f0t, "f1t": f1t, "f2t": f2t, "f3t": f3t,
             "w": w, "b": b}
        )
    return in_maps


LAST_EXEC_NS = None


def kernel(x, weight, f0, f1, f2, f3, bias):
    global LAST_EXEC_NS
    from concourse.bass_utils import run_bass_kernel_spmd

    nc = _get_program()
    in_maps = _host_prep(x, weight, f0, f1, f2, f3, bias)
    trace = bool(int(os.environ.get("BASS_KERNEL_TRACE", "0")))
    res = run_bass_kernel_spmd(nc, in_maps, list(range(_NCORES)), trace=trace)
    LAST_EXEC_NS = res.exec_time_ns
    out = np.concatenate([res.results[c]["out"] for c in range(_NCORES)], axis=0)
    return np.ascontiguousarray(out.astype(np.float32, copy=False))


# revision 12
# speedup vs baseline: 1.1461x; 1.1461x over previous
"""CP tensor-regression-layer kernel for Trainium2 (8 NeuronCores).

Computation (matches the reference einsum pair):
    t[b, r]  = sum_{i,j,k} x[b,i,j,k] * f0[i,r] * f1[j,r] * f2[k,r]
    out[b,c] = sum_r t[b,r] * weight[r] * f3[c,r] + bias[0]

Strategy: data-parallel over the batch dim (32 batches per core, CP
factors replicated).  Per core the big contraction is restructured as
    z[r, b, k] = sum_{ij} (f0[i,r]*f1[j,r]*weight[r]) * x[b, ij, k]
which is a K=2304 matmul against the Khatri-Rao product of f0 and f1,
run as 18 K-chunks of 128 partitions at full PE rate (float32r).  The
remaining k-contraction against f2 runs on the vector engine, and the
class projection against f3^T is one small matmul.  x is pre-permuted
on the host so every DMA is 128 partitions x 6 KiB contiguous runs —
the kernel is HBM-bandwidth bound on loading x (~14.2 MB/core).
"""

import os

import numpy as np

_B, _M1, _M2, _M3, _C, _R = 256, 48, 48, 48, 1000, 64
_NCORES = 8
_BL = _B // _NCORES          # 32 batches per core
_IJ = _M1 * _M2              # 2304 contraction size (i,j fused)
_NCH = _IJ // 128            # 18 K-chunks of 128 partitions
_KB = _BL * _M3              # 1536 moving columns (b,k fused)
_SL = 512                    # matmul slice width (one PSUM bank, fp32)

_cache = {}


def _split_excess_waits(nc, mybir, max_waits=1):
    """Walrus in this container rejects >1 sync-wait per instruction
    ("Too many sync wait commands").  Move excess waits onto chained
    NoOps inserted just before the offending instruction (same engine,
    so program order preserves the gating)."""
    for bb in nc.m.functions[0].blocks:
        insts = bb.instructions
        i = 0
        while i < len(insts):
            inst = insts[i]
            si = getattr(inst, "sync_info", None)
            waits = list(si.on_wait) if si is not None and si.on_wait else []
            if len(waits) > max_waits:
                rest, keep = waits[:-max_waits], waits[-max_waits:]
                pos = i
                for j in range(0, len(rest), max_waits):
                    nop = mybir.InstNoOp(
                        name=f"I-waitsplit-{nc.next_id()}",
                        engine=inst.engine,
                        ins=[],
                        outs=[],
                        sync_info=mybir.SyncInfo(
                            on_wait=list(rest[j : j + max_waits]), on_update=[]
                        ),
                    )
                    nc.register_instruction(nop)
                    insts.insert(pos, nop)
                    pos += 1
                    i += 1
                si.on_wait = keep
            i += 1


def _bcast(ap, bass, shape3):
    """AP broadcast helper: make a 3D view with a stride-0 middle dim."""
    try:
        return ap.unsqueeze(1).broadcast_to(shape3)
    except Exception:
        a = ap.ap
        return bass.AP(
            tensor=ap.tensor,
            offset=ap.offset,
            ap=[list(a[0]), [0, shape3[1]], list(a[1])],
        )


def _build_program():
    import concourse.bass as bass
    import concourse.tile as tile
    from concourse import mybir

    f32 = mybir.dt.float32
    f32r = mybir.dt.float32r

    nc = bass.Bass("TRN2", target_bir_lowering=False, debug=False,
                   num_devices=_NCORES)

    x_d = nc.dram_tensor("x", [128, _NCH, _BL, _M3], f32r, kind="ExternalInput")
    f0t_d = nc.dram_tensor("f0t", [_R, _M1], f32, kind="ExternalInput")
    f1t_d = nc.dram_tensor("f1t", [_R, _M2], f32, kind="ExternalInput")
    f2t_d = nc.dram_tensor("f2t", [_R, _M3], f32, kind="ExternalInput")
    f3t_d = nc.dram_tensor("f3t", [_R, _C], f32r, kind="ExternalInput")
    w_d = nc.dram_tensor("w", [_R, 1], f32, kind="ExternalInput")
    b_d = nc.dram_tensor("b", [1, 1], f32, kind="ExternalInput")
    out_d = nc.dram_tensor("out", [_BL, _C], f32, kind="ExternalOutput")
    ident_d = nc.inline_tensor(np.eye(_R, dtype=np.float32), name="ident64")

    with tile.TileContext(nc) as tc:
        with (
            tc.tile_pool(name="consts", bufs=1) as consts,
            tc.tile_pool(name="xp", bufs=_NCH) as xp,
            tc.tile_pool(name="work", bufs=1) as work,
            tc.tile_pool(name="pz", bufs=1, space=bass.MemorySpace.PSUM) as pz,
            tc.tile_pool(name="pt", bufs=3, space=bass.MemorySpace.PSUM) as pt,
            tc.tile_pool(name="po", bufs=1, space=bass.MemorySpace.PSUM) as po,
        ):
            # ---- constants / factor prep (overlaps the x DMA stream) ----
            f0t = consts.tile([_R, _M1], f32)
            nc.sync.dma_start(out=f0t[:], in_=f0t_d[:])
            f1t = consts.tile([_R, _M2], f32)
            nc.sync.dma_start(out=f1t[:], in_=f1t_d[:])
            f2t = consts.tile([_R, _M3], f32)
            nc.sync.dma_start(out=f2t[:], in_=f2t_d[:])
            f3t = consts.tile([_R, _C], f32r)
            nc.sync.dma_start(out=f3t[:], in_=f3t_d[:])
            wsb = consts.tile([_R, 1], f32)
            nc.gpsimd.dma_start(out=wsb[:], in_=w_d[:])
            idn = consts.tile([_R, _R], f32)
            nc.sync.dma_start(out=idn[:], in_=ident_d[:])
            bsb = consts.tile([_BL, 1], f32)
            b_ap = b_d[:]
            nc.gpsimd.dma_start(
                out=bsb[:],
                in_=bass.AP(tensor=b_ap.tensor, offset=b_ap.offset,
                            ap=[[0, _BL], [0, 1]]),
            )

            # KR01T[r, i, j] = f0[i,r] * f1[j,r] * weight[r]
            f1tw = consts.tile([_R, _M2], f32)
            nc.vector.tensor_scalar_mul(f1tw[:], f1t[:], wsb[:])
            krt = consts.tile([_R, _M1, _M2], f32)
            in0 = f0t[:].unsqueeze(2).broadcast_to((_R, _M1, _M2))
            in1 = _bcast(f1tw[:], bass, (_R, _M1, _M2))
            nc.vector.tensor_mul(krt[:], in0, in1)

            # Transpose KR to put ij on partitions: kr[p, m, r] = KR[128m+p, r]
            kr = consts.tile([128, _NCH, _R], f32r)
            krt_flat = krt[:].rearrange("r i j -> r (i j)")
            for m in range(_NCH):
                pkr = pt.tile([128, _R], f32)
                nc.tensor.transpose(
                    pkr[:], krt_flat[:, m * 128 : (m + 1) * 128], idn[:]
                )
                nc.vector.tensor_copy(kr[:, m, :], pkr[:])

            # ---- main contraction: z[r, (b,k)] = sum_ij KR[ij,r] x[b,ij,k] ----
            z = pz.tile([_R, _KB], f32)
            for m in range(_NCH):
                xm = xp.tile([128, _BL, _M3], f32r, tag="x")
                # Alternate the two HWDGE rings (SP / ACT) so packet
                # generation for consecutive chunks overlaps.
                dma_eng = nc.sync if m % 2 == 0 else nc.scalar
                dma_eng.dma_start(out=xm[:], in_=x_d[:, m])
                xm_f = xm[:].rearrange("p b k -> p (b k)")
                for s in range(_KB // _SL):
                    nc.tensor.matmul(
                        z[:, s * _SL : (s + 1) * _SL],
                        lhsT=kr[:, m, :],
                        rhs=xm_f[:, s * _SL : (s + 1) * _SL],
                        start=(m == 0),
                        stop=(m == _NCH - 1),
                    )

            # ---- k-contraction against f2 on the vector engine ----
            zf = work.tile([_R, _BL, _M3], f32)
            z3 = z[:].rearrange("r (b k) -> r b k", k=_M3)
            f2b = _bcast(f2t[:], bass, (_R, _BL, _M3))
            nc.vector.tensor_mul(zf[:], z3, f2b)
            tsb = work.tile([_R, _BL], f32r)
            # DVE accumulates in fp32 internally; only the output value is
            # rounded to f32r (which the stage-4 f32r matmul requires).
            with nc.allow_low_precision(reason="f32r rounding for PE matmul"):
                nc.vector.reduce_sum(tsb[:], zf[:], axis=mybir.AxisListType.X)

            # ---- class projection: out[b, c] = sum_r t[r,b] f3T[r,c] + bias ----
            op = po.tile([_BL, _C], f32)
            for n0, n1 in ((0, _SL), (_SL, _C)):
                nc.tensor.matmul(
                    op[:, n0:n1],
                    lhsT=tsb[:],
                    rhs=f3t[:, n0:n1],
                    start=True,
                    stop=True,
                )
            osb = work.tile([_BL, _C], f32)
            nc.vector.tensor_scalar_add(osb[:], op[:], bsb[:])
            nc.sync.dma_start(out=out_d[:], in_=osb[:])

    _split_excess_waits(nc, mybir)
    return nc


def _get_program():
    if "nc" not in _cache:
        _cache["nc"] = _build_program()
    return _cache["nc"]


def _host_prep(x, weight, f0, f1, f2, f3, bias):
    """Shard x over cores (batch dim) in a DMA-friendly layout, and
    transpose the small factor matrices (layout only, plus reshapes)."""
    x = np.ascontiguousarray(np.asarray(x, dtype=np.float32))
    f0t = np.ascontiguousarray(np.asarray(f0, np.float32).T)
    f1t = np.ascontiguousarray(np.asarray(f1, np.float32).T)
    f2t = np.ascontiguousarray(np.asarray(f2, np.float32).T)
    f3t = np.ascontiguousarray(np.asarray(f3, np.float32).T)
    w = np.ascontiguousarray(np.asarray(weight, np.float32).reshape(_R, 1))
    b = np.ascontiguousarray(np.asarray(bias, np.float32).reshape(1, 1))
    in_maps = []
    for c in range(_NCORES):
        xc = x[c * _BL : (c + 1) * _BL]
        # [b, ij, k] -> [p, m, b, k] with ij = 128*m + p
        xd = np.ascontiguousarray(
            xc.reshape(_BL, _NCH, 128, _M3).transpose(2, 1, 0, 3)
        )
        in_maps.append(
            {"x": xd, "f0t": f0t, "f1t": f1t, "f2t": f2t, "f3t": f3t,
             "w": w, "b": b}
        )
    return in_maps


LAST_EXEC_NS = None


def kernel(x, weight, f0, f1, f2, f3, bias):
    global LAST_EXEC_NS
    from concourse.bass_utils import run_bass_kernel_spmd

    nc = _get_program()
    in_maps = _host_prep(x, weight, f0, f1, f2, f3, bias)
    trace = bool(int(os.environ.get("BASS_KERNEL_TRACE", "0")))
    res = run_bass_kernel_spmd(nc, in_maps, list(range(_NCORES)), trace=trace)
    LAST_EXEC_NS = res.exec_time_ns
    out = np.concatenate([res.results[c]["out"] for c in range(_NCORES)], axis=0)
    return np.ascontiguousarray(out.astype(np.float32, copy=False))
